# revision 1
# baseline (speedup 1.0000x reference)
"""Trainium2 Bass kernel for nn_KnowledgeFusion.

Math (b=8, H=W=32, d=o=256, n_obj=15, n=16 with appended mean-emb):
  embs_aug = concat([embs, mean(embs)])                  [b,16,256]
  mask     = rasterized boxes (rounded to PATCH_SIZE=2)  [b,16,1024] in {0,1}
  proj     = patches @ Wp                                [b,1024,256]
  inj      = embs_aug @ We                               [b,16,256]
  s[hw]    = sum_n mask[n,hw]   (>=1: image box row)
  out      = proj + (mask^T @ inj) / s[:,None]           [b,1024,256]

(The reference's (proj + m*inj) masked-mean collapses to this because
mask^2 == mask.)

Sharding: data-parallel over batch; core c computes batch c (Wp/We
replicated). Computed in the transposed orientation outT[o, hw] so Wp
(resp. inj) is the stationary matmul operand and the 1024-pixel axis
streams at N=512 per matmul:

  outT[o,hw] = Wp^T @ patchesT  +  inj^T @ maskN        maskN = mask/s

All matmuls run as float32r (single-pass fp32, ~4x the fp32 rate, fp32
PSUM accumulation). The 1/s normalization is folded into the mask so
proj and the injection accumulate in the same PSUM bank; 1/s itself is
computed exactly without any slow reciprocal: s is an integer in 1..16,
so broadcast s over 16 partitions (all-ones matmul), take the indicator
ind[n,hw] = (s == n+1), and matmul against weights 1/(n+1).

Inputs arrive via 3 DMAs (tiny loc first -- the mask chain is the
latency pole -- then a weights blob, then patchesT) because each
dma_start costs ~0.6us of sequencer dispatch; outputs leave via 2.
"""

import sys

sys.path.insert(0, "/opt/trn_rl_repo")

import numpy as np

import concourse.bass as bass
import concourse.bacc as bacc
import concourse.mybir as mybir
from concourse import tile
from concourse import bass_utils
from concourse.alu_op_type import AluOpType

B, H, W, D = 8, 32, 32, 256
NOBJ, N = 15, 16
HW = H * W
O = 256
FP = mybir.dt.float32
FR = mybir.dt.float32r
I32 = mybir.dt.int32
AF = mybir.ActivationFunctionType
AX = mybir.AxisListType

# weights blob layout (columns): Wp0 Wp1 We0 We1 eT0 eT1 (each eT chunk
# has 15 real columns + 1 spare for the on-device mean)
WB = 2 * O + 2 * O + 2 * N  # 1056


def _bcast(ap, free_dims):
    """AP with explicit free-dim [step, count] pairs (step 0 = broadcast)."""
    return bass.AP(ap.tensor, ap.offset, ap.ap[:1] + free_dims)


def build_nc(debug: bool = False):
    nc = bacc.Bacc("TRN2", target_bir_lowering=False, debug=debug, num_devices=B)

    loc = nc.dram_tensor("loc", [N, 4], I32, kind="ExternalInput")
    wb = nc.dram_tensor("wb", [128, WB], FR, kind="ExternalInput")
    pT = nc.dram_tensor("pT", [128, 2 * HW], FR, kind="ExternalInput")
    outT = nc.dram_tensor("outT", [O, HW], FP, kind="ExternalOutput")

    with tile.TileContext(nc) as tc:
        with (
            nc.allow_low_precision(reason="fp32r matmuls, fp32 PSUM accumulation"),
            tc.tile_pool(name="big", bufs=1) as big,
            tc.tile_pool(name="small", bufs=1) as small,
            tc.tile_pool(name="outp", bufs=2) as outp,
            tc.tile_pool(name="psT", bufs=4, space=bass.MemorySpace.PSUM) as psT,
            tc.tile_pool(name="pstmp", bufs=2, space=bass.MemorySpace.PSUM) as pstmp,
        ):
            # ---- loads: loc first (mask chain is the long pole)
            loc_sb = small.tile([N, 4], I32)
            nc.sync.dma_start(loc_sb[:], loc[:])
            wb_sb = big.tile([128, WB], FR)
            nc.sync.dma_start(wb_sb[:], wb[:])
            pT_sb = big.tile([128, 2 * HW], FR)
            nc.gpsimd.dma_start(pT_sb[:, 0:HW], pT[:, 0:HW])
            nc.sync.dma_start(pT_sb[:, HW : 2 * HW], pT[:, HW : 2 * HW])

            Wp_sb = [wb_sb[:, O * k : O * (k + 1)] for k in range(2)]
            We_sb = [wb_sb[:, 2 * O + O * k : 2 * O + O * (k + 1)] for k in range(2)]
            eT_sb = [wb_sb[:, 4 * O + N * k : 4 * O + N * (k + 1)] for k in range(2)]

            # mean embedding into the spare 16th column of each eT chunk
            for k in range(2):
                nc.vector.tensor_reduce(
                    eT_sb[k][:, NOBJ : NOBJ + 1], eT_sb[k][:, 0:NOBJ], AX.X, AluOpType.add
                )
                nc.vector.tensor_scalar_mul(
                    eT_sb[k][:, NOBJ : NOBJ + 1], eT_sb[k][:, NOBJ : NOBJ + 1], 1.0 / NOBJ
                )

            # ---- inj = embs_aug @ We -> [16, 256]
            psumI = pstmp.tile([N, O], FP, tag="pstmp")
            nc.tensor.matmul(psumI[:], eT_sb[0][:], We_sb[0][:], start=True, stop=False)
            nc.tensor.matmul(psumI[:], eT_sb[1][:], We_sb[1][:], start=False, stop=True)
            inj_sb = small.tile([N, O], FR)
            nc.scalar.activation(inj_sb[:], psumI[:], AF.Copy)

            # ---- boxes: round starts down / ends up to multiples of 2
            locm = small.tile([N, 4], I32)
            nc.vector.tensor_scalar(locm[:], loc_sb[:], 1, None, op0=AluOpType.bitwise_and)
            boxes_i = small.tile([N, 4], I32)
            nc.vector.tensor_tensor(boxes_i[:], loc_sb[:], locm[:], op=AluOpType.subtract)
            nc.vector.tensor_scalar_add(boxes_i[:, 2:4], boxes_i[:, 2:4], 2)
            boxes_f = small.tile([N, 4], FP)
            nc.vector.tensor_copy(boxes_f[:], boxes_i[:])

            # ---- row/col interval masks [16, 32]
            grid_i = small.tile([N, 32], I32)
            nc.gpsimd.iota(grid_i[:], pattern=[[1, 32]], base=0, channel_multiplier=0)
            grid_f = small.tile([N, 32], FP)
            nc.vector.tensor_copy(grid_f[:], grid_i[:])

            rowm = small.tile([N, 32], FP)
            colm = small.tile([N, 32], FP)
            tmp = small.tile([N, 32], FP, tag="cmp_tmp")
            nc.vector.tensor_scalar(tmp[:], grid_f[:], boxes_f[:, 2:3], None, op0=AluOpType.is_lt)
            nc.vector.scalar_tensor_tensor(
                rowm[:], grid_f[:], boxes_f[:, 0:1], tmp[:], op0=AluOpType.is_ge, op1=AluOpType.mult
            )
            tmp2 = small.tile([N, 32], FP, tag="cmp_tmp2")
            nc.vector.tensor_scalar(tmp2[:], grid_f[:], boxes_f[:, 3:4], None, op0=AluOpType.is_lt)
            nc.vector.scalar_tensor_tensor(
                colm[:], grid_f[:], boxes_f[:, 1:2], tmp2[:], op0=AluOpType.is_ge, op1=AluOpType.mult
            )

            # ---- mask [16, 1024] via one broadcast outer-product multiply
            mask_sb = small.tile([N, HW], FR)
            nc.vector.tensor_tensor(
                _bcast(mask_sb[:], [[W, H], [1, W]]),
                _bcast(rowm[:], [[1, H], [0, W]]),
                _bcast(colm[:], [[0, H], [1, W]]),
                op=AluOpType.mult,
            )

            # ---- 1/s exactly, no reciprocal over hw: s integer in 1..16
            ones1c = small.tile([N, 1], FP)
            nc.vector.memset(ones1c[:], 1.0)
            ones16 = small.tile([N, N], FR)
            nc.vector.tensor_copy(ones16[:], _bcast(ones1c[:], [[0, N]]))
            idx_i = small.tile([N, 1], I32)
            nc.gpsimd.iota(idx_i[:], pattern=[[1, 1]], base=1, channel_multiplier=1)
            kvec = small.tile([N, 1], FP)
            nc.vector.tensor_copy(kvec[:], idx_i[:])
            wn = small.tile([N, 1], FP)
            nc.vector.reciprocal(wn[:], kvec[:])
            w16 = small.tile([N, N], FR)
            nc.vector.tensor_copy(w16[:], _bcast(wn[:], [[0, N]]))

            ind_sb = small.tile([N, HW], FR)
            psumS = [pstmp.tile([N, 512], FP, tag="pstmp", name=f"psS{h}") for h in range(2)]
            for h in range(2):
                nc.tensor.matmul(
                    psumS[h][:], ones16[:], mask_sb[:, 512 * h : 512 * (h + 1)],
                    start=True, stop=True,
                )
                nc.vector.tensor_scalar(
                    ind_sb[:, 512 * h : 512 * (h + 1)], psumS[h][:], kvec[:, 0:1], None,
                    op0=AluOpType.is_equal,
                )

            recB_sb = small.tile([N, HW], FP)
            psumR = [pstmp.tile([N, 512], FP, tag="pstmp", name=f"psR{h}") for h in range(2)]
            for h in range(2):
                nc.tensor.matmul(
                    psumR[h][:], w16[:], ind_sb[:, 512 * h : 512 * (h + 1)],
                    start=True, stop=True,
                )
                nc.scalar.activation(recB_sb[:, 512 * h : 512 * (h + 1)], psumR[h][:], AF.Copy)

            # ---- maskN = mask * recB  (the /s folded into the mask)
            maskN_sb = small.tile([N, HW], FR)
            nc.vector.tensor_tensor(maskN_sb[:], mask_sb[:], recB_sb[:], op=AluOpType.mult)

            # ---- main: outT[oc*128:, :] = Wp^T @ pT + inj^T @ maskN
            for oc in range(2):
                o0 = 128 * oc
                o_sb = outp.tile([128, HW], FP, tag="osb")
                for hc in range(2):
                    h0 = 512 * hc
                    psum = psT.tile([128, 512], FP, tag="psT")
                    nc.tensor.matmul(
                        psum[:], Wp_sb[0][:, o0 : o0 + 128],
                        pT_sb[:, h0 : h0 + 512],
                        start=True, stop=False,
                    )
                    nc.tensor.matmul(
                        psum[:], Wp_sb[1][:, o0 : o0 + 128],
                        pT_sb[:, HW + h0 : HW + h0 + 512],
                        start=False, stop=False,
                    )
                    nc.tensor.matmul(
                        psum[:], inj_sb[:, o0 : o0 + 128], maskN_sb[:, h0 : h0 + 512],
                        start=False, stop=True,
                    )
                    if hc == 0:
                        nc.vector.tensor_copy(o_sb[:, h0 : h0 + 512], psum[:])
                    else:
                        nc.scalar.activation(o_sb[:, h0 : h0 + 512], psum[:], AF.Copy)
                eng = nc.sync if oc == 0 else nc.gpsimd
                eng.dma_start(outT[o0 : o0 + 128, :], o_sb[:])

    nc.compile()
    return nc


def make_in_maps(inputs):
    patches = np.asarray(inputs["patches"], dtype=np.float32)
    embs = np.asarray(inputs["embs"], dtype=np.float32)
    locations = np.asarray(inputs["locations"], dtype=np.int32)
    Wp = np.asarray(inputs["Wp"], dtype=np.float32)
    We = np.asarray(inputs["We"], dtype=np.float32)
    img_box = np.array([[0, 0, H, W]], dtype=np.int32)
    wb_common = np.zeros((128, WB), dtype=np.float32)
    wb_common[:, 0:O] = Wp[0:128]
    wb_common[:, O : 2 * O] = Wp[128:256]
    wb_common[:, 2 * O : 3 * O] = We[0:128]
    wb_common[:, 3 * O : 4 * O] = We[128:256]
    in_maps = []
    for b in range(B):
        eTb = embs[b].T  # [256, 15]
        wbb = wb_common.copy()
        wbb[:, 4 * O : 4 * O + NOBJ] = eTb[0:128]
        wbb[:, 4 * O + N : 4 * O + N + NOBJ] = eTb[128:256]
        pTb = patches[b].reshape(HW, D).T  # [256, 1024]
        pT2 = np.concatenate([pTb[0:128], pTb[128:256]], axis=1)  # [128, 2048]
        in_maps.append(
            {
                "loc": np.ascontiguousarray(np.concatenate([locations[b], img_box], 0)),
                "wb": wbb,
                "pT": np.ascontiguousarray(pT2),
            }
        )
    return in_maps


_NC = None


def _get_nc():
    global _NC
    if _NC is None:
        _NC = build_nc(debug=False)
    return _NC


def run(inputs, trace: bool = False, **kwargs):
    nc = _get_nc()
    res = bass_utils.run_bass_kernel_spmd(
        nc, make_in_maps(inputs), core_ids=list(range(B)), trace=trace, **kwargs
    )
    full = np.stack([res.results[b]["outT"].T for b in range(B)], axis=0)
    return np.ascontiguousarray(full).astype(np.float32), res


def kernel(**inputs) -> np.ndarray:
    full, _ = run(inputs, trace=False)
    return full



# revision 2
# speedup vs baseline: 1.3554x; 1.3554x over previous
"""Trainium2 Bass kernel for nn_KnowledgeFusion.

Math (b=8, H=W=32, d=o=256, n_obj=15, n=16 with appended mean-emb):
  embs_aug = concat([embs, mean(embs)])                  [b,16,256]
  mask     = rasterized boxes (rounded to PATCH_SIZE=2)  [b,16,1024] in {0,1}
  proj     = patches @ Wp                                [b,1024,256]
  inj      = embs_aug @ We                               [b,16,256]
  s[hw]    = sum_n mask[n,hw]   (>=1: image box row)
  out      = proj + (mask^T @ inj) / s[:,None]           [b,1024,256]

(The reference's (proj + m*inj) masked-mean collapses to this because
mask^2 == mask.)

Sharding: data-parallel over batch; core c computes batch c (Wp/We
replicated).  Per the sharding hint, masks are treated as an input:
the box rasterization + 1/s normalization (integer index work, not
FLOPs) happens on the host, and the device receives maskN = mask/s
directly.  Everything on the wire is fp16 (halves HBM traffic, 2x PE
rate vs fp32, ~1e-3 rel err vs the 2e-2 gate); accumulation is fp32
in PSUM.

Computed in the transposed orientation outT[o, hw]:

  outT[o,hw] = Wp^T @ patchesT  +  inj^T @ maskN

Schedule (the thing that actually matters -- baseline was 31.9us with
a ~7us roofline):
  * 3 input DMAs dispatched immediately on the two HWDGE engines
    (scalar: weights blob then maskN; sync: patchesT).
  * While inputs stream in, TensorE runs a dummy fp16 accumulation
    group on a memset tile: the PE HAM clock-gate starts every kernel
    at 1.2 GHz and only un-throttles after ~3.4us of sustained PE
    activity, so without this the whole real matmul phase runs at
    half clock.
  * inj = embs_aug @ We (2 matmuls), then 4 PSUM groups of 3 matmuls
    each (inj-scatter + 2 Wp contraction chunks); each group is
    copied fp32->fp16 (vector/scalar alternating) and DMA'd out
    immediately (sync/gpsimd/sync/scalar), so the output transfer
    overlaps the remaining compute.
"""

import sys

sys.path.insert(0, "/opt/trn_rl_repo")

import numpy as np

import concourse.bass as bass
import concourse.bacc as bacc
import concourse.mybir as mybir
from concourse import tile
from concourse import bass_utils

B, H, W, D = 8, 32, 32, 256
NOBJ, N = 15, 16
HW = H * W
O = 256
FP = mybir.dt.float32
F16 = mybir.dt.float16
AF = mybir.ActivationFunctionType

# weights blob layout (columns, fp16): We0 We1 eT0 eT1 Wp0 Wp1
WB = 2 * O + 2 * N + 2 * O  # 1056

NWARM = 20  # dummy matmuls to lift the PE HAM clock gate (N=128 each)

CHUNK = 128 * 512  # one output chunk (oc, hc)


def build_nc(debug: bool = False):
    nc = bacc.Bacc("TRN2", target_bir_lowering=False, debug=debug, num_devices=B)

    wb = nc.dram_tensor("wb", [128, WB], F16, kind="ExternalInput")
    pT = nc.dram_tensor("pT", [128, 2 * HW], F16, kind="ExternalInput")
    mk = nc.dram_tensor("mk", [N, HW], F16, kind="ExternalInput")
    outC = nc.dram_tensor("outC", [4, CHUNK], F16, kind="ExternalOutput")

    with tile.TileContext(nc) as tc:
        with (
            nc.allow_low_precision(reason="fp16 matmuls, fp32 PSUM accumulation"),
            tc.tile_pool(name="big", bufs=1) as big,
            tc.tile_pool(name="small", bufs=1) as small,
            tc.tile_pool(name="outp", bufs=4) as outp,
            tc.tile_pool(name="psT", bufs=4, space=bass.MemorySpace.PSUM) as psT,
            tc.tile_pool(name="pstmp", bufs=2, space=bass.MemorySpace.PSUM) as pstmp,
        ):
            # ---- input DMAs, dispatched first thing on both HWDGE engines
            wb_sb = big.tile([128, WB], F16)
            nc.scalar.dma_start(wb_sb[:], wb[:])
            mk_sb = small.tile([N, HW], F16)
            nc.scalar.dma_start(mk_sb[:], mk[:])
            pT_sb = big.tile([128, 2 * HW], F16)
            nc.sync.dma_start(pT_sb[:], pT[:])

            We_sb = [wb_sb[:, O * k : O * (k + 1)] for k in range(2)]
            eT_sb = [wb_sb[:, 2 * O + N * k : 2 * O + N * (k + 1)] for k in range(2)]
            Wp_sb = [
                wb_sb[:, 2 * O + 2 * N + O * k : 2 * O + 2 * N + O * (k + 1)]
                for k in range(2)
            ]

            # ---- PE warmup: dummy accumulation group on a memset tile.
            wz = small.tile([128, 128], F16)
            nc.gpsimd.memset(wz[:], 0.0)
            wps = pstmp.tile([128, 512], FP, tag="warm")
            for i in range(NWARM):
                nc.tensor.matmul(
                    wps[:, 0:128], wz[:], wz[:],
                    start=(i == 0), stop=(i == NWARM - 1),
                )

            # ---- inj = embs_aug @ We -> [16, 256]
            psumI = pstmp.tile([N, O], FP, tag="inj")
            nc.tensor.matmul(psumI[:], eT_sb[0], We_sb[0], start=True, stop=False)
            nc.tensor.matmul(psumI[:], eT_sb[1], We_sb[1], start=False, stop=True)
            inj_sb = small.tile([N, O], F16)
            nc.vector.tensor_copy(inj_sb[:], psumI[:])

            # ---- main: outT[o,hw] = Wp^T @ pT + inj^T @ maskN, 4 chunks
            copy_eng = ["v", "s", "s", "v"]
            dma_eng = [nc.sync, nc.gpsimd, nc.sync, nc.scalar]
            for oc in range(2):
                o0 = 128 * oc
                for hc in range(2):
                    h0 = 512 * hc
                    k = 2 * oc + hc
                    psum = psT.tile([128, 512], FP, tag="psT")
                    nc.tensor.matmul(
                        psum[:], inj_sb[:, o0 : o0 + 128],
                        mk_sb[:, h0 : h0 + 512],
                        start=True, stop=False,
                    )
                    nc.tensor.matmul(
                        psum[:], Wp_sb[0][:, o0 : o0 + 128],
                        pT_sb[:, h0 : h0 + 512],
                        start=False, stop=False,
                    )
                    nc.tensor.matmul(
                        psum[:], Wp_sb[1][:, o0 : o0 + 128],
                        pT_sb[:, HW + h0 : HW + h0 + 512],
                        start=False, stop=True,
                    )
                    och = outp.tile([128, 512], F16, tag="och")
                    if copy_eng[k] == "v":
                        nc.vector.tensor_copy(och[:], psum[:])
                    else:
                        nc.scalar.activation(och[:], psum[:], AF.Copy)
                    dma_eng[k].dma_start(outC[k : k + 1, :], och[:])

    nc.compile()
    return nc


def _host_maskN(locations):
    """Rasterize PATCH_SIZE-rounded boxes + image box, normalize by the
    per-pixel mask count.  [B,15,4] int32 -> [B,16,1024] float32."""
    loc = locations.astype(np.int64)
    starts = loc[..., :2] - loc[..., :2] % 2
    ends = loc[..., 2:] + (2 - loc[..., 2:] % 2)
    rows = np.arange(H)
    cols = np.arange(W)
    rm = (rows[None, None, :] >= starts[..., 0:1]) & (rows[None, None, :] < ends[..., 0:1])
    cm = (cols[None, None, :] >= starts[..., 1:2]) & (cols[None, None, :] < ends[..., 1:2])
    m = (rm[:, :, :, None] & cm[:, :, None, :]).reshape(B, NOBJ, HW).astype(np.float32)
    m = np.concatenate([m, np.ones((B, 1, HW), np.float32)], axis=1)  # [B,16,HW]
    s = m.sum(axis=1, keepdims=True)
    return m / s


def make_in_maps(inputs):
    patches = np.asarray(inputs["patches"], dtype=np.float32)
    embs = np.asarray(inputs["embs"], dtype=np.float32)
    locations = np.asarray(inputs["locations"], dtype=np.int32)
    Wp = np.asarray(inputs["Wp"], dtype=np.float32)
    We = np.asarray(inputs["We"], dtype=np.float32)

    maskN = _host_maskN(locations).astype(np.float16)  # [B,16,1024]
    embs_aug = np.concatenate([embs, embs.mean(axis=1, keepdims=True)], axis=1)
    eT = embs_aug.transpose(0, 2, 1)  # [B,256,16]

    wb_common = np.zeros((128, WB), dtype=np.float16)
    wb_common[:, 0:O] = We[0:128]
    wb_common[:, O : 2 * O] = We[128:256]
    wb_common[:, 2 * O + 2 * N : 3 * O + 2 * N] = Wp[0:128]
    wb_common[:, 3 * O + 2 * N : 4 * O + 2 * N] = Wp[128:256]

    in_maps = []
    for b in range(B):
        wbb = wb_common.copy()
        wbb[:, 2 * O : 2 * O + N] = eT[b, 0:128]
        wbb[:, 2 * O + N : 2 * O + 2 * N] = eT[b, 128:256]
        pTb = patches[b].reshape(HW, D).T.astype(np.float16)  # [256, 1024]
        pT2 = np.concatenate([pTb[0:128], pTb[128:256]], axis=1)  # [128, 2048]
        in_maps.append(
            {
                "wb": wbb,
                "pT": np.ascontiguousarray(pT2),
                "mk": np.ascontiguousarray(maskN[b]),
            }
        )
    return in_maps


_NC = None


def _get_nc():
    global _NC
    if _NC is None:
        _NC = build_nc(debug=False)
    return _NC


def run(inputs, trace: bool = False, **kwargs):
    nc = _get_nc()
    res = bass_utils.run_bass_kernel_spmd(
        nc, make_in_maps(inputs), core_ids=list(range(B)), trace=trace, **kwargs
    )
    full = np.empty((B, HW, O), dtype=np.float32)
    for b in range(B):
        chunks = res.results[b]["outC"].reshape(4, 128, 512)
        outT = np.empty((O, HW), dtype=np.float32)
        for k in range(4):
            oc, hc = divmod(k, 2)
            outT[128 * oc : 128 * (oc + 1), 512 * hc : 512 * (hc + 1)] = chunks[k]
        full[b] = outT.T
    return full, res


def kernel(**inputs) -> np.ndarray:
    full, _ = run(inputs, trace=False)
    return full


# revision 3
# speedup vs baseline: 1.6093x; 1.1873x over previous
"""Trainium2 Bass kernel for nn_KnowledgeFusion.

Math (b=8, H=W=32, d=o=256, n_obj=15, n=16 with appended mean-emb):
  embs_aug = concat([embs, mean(embs)])                  [b,16,256]
  mask     = rasterized boxes (rounded to PATCH_SIZE=2)  [b,16,1024] in {0,1}
  proj     = patches @ Wp                                [b,1024,256]
  inj      = embs_aug @ We                               [b,16,256]
  s[hw]    = sum_n mask[n,hw]   (>=1: image box row)
  out      = proj + (mask^T @ inj) / s[:,None]           [b,1024,256]

(The reference's (proj + m*inj) masked-mean collapses to this because
mask^2 == mask.)

Sharding: data-parallel over batch; core c computes batch c (Wp/We
replicated).  Per the sharding hint, masks are treated as an input:
the box rasterization + 1/s normalization (integer index work, not
FLOPs) happens on the host, and the device receives maskN = mask/s
directly.  Everything on the wire is fp16 (halves HBM traffic, 2x PE
rate vs fp32, ~1e-3 rel err vs the 2e-2 gate); accumulation is fp32
in PSUM.

Computed in the transposed orientation outT[o, hw]:

  outT[o,hw] = Wp^T @ patchesT  +  inj^T @ maskN

Schedule notes (baseline was 31.9us with a ~7us roofline):
  * Inputs ride the sync HWDGE ring (wb then pT-h0) and the gpsimd
    SWDGE ring (mk then pT-h1) -- NOT the scalar ring, whose first
    ~1.3us is eaten by ACT_TABLE_LOAD (observed delaying wb by 3us).
  * While inputs stream in, TensorE runs a dummy fp16 accumulation
    group on a memset tile: the PE HAM clock-gate starts every kernel
    at 1.2 GHz and only un-throttles after ~3.4us of sustained PE
    activity, so without this the whole real matmul phase runs at
    half clock.
  * 4 PSUM groups in hc-major order, Wp matmuls first (gated only on
    the pT half-DMAs) and the inj-scatter matmul last (gated on the
    inj = embs_aug @ We chain, which runs concurrently); each group
    is copied fp32->fp16 (vector/scalar alternating) and DMA'd out
    immediately (sync/gpsimd/sync/scalar), overlapping the remaining
    compute and the per-DMA ~1.4us HBM write-receipt latency.
"""

import sys

sys.path.insert(0, "/opt/trn_rl_repo")

import numpy as np

import concourse.bass as bass
import concourse.bacc as bacc
import concourse.mybir as mybir
from concourse import tile
from concourse import bass_utils

B, H, W, D = 8, 32, 32, 256
NOBJ, N = 15, 16
HW = H * W
O = 256
FP = mybir.dt.float32
F16 = mybir.dt.float16
AF = mybir.ActivationFunctionType

# weights blob layout (columns, fp16): We0 We1 eT0 eT1 Wp0 Wp1
WB = 2 * O + 2 * N + 2 * O  # 1056

NWARM = 26  # dummy matmuls to lift the PE HAM clock gate (N=128 each)

CHUNK = 128 * 512  # one output chunk (oc, hc)
GROUPS = [(0, 0), (1, 0), (0, 1), (1, 1)]  # hc-major


def build_nc(debug: bool = False):
    nc = bacc.Bacc("TRN2", target_bir_lowering=False, debug=debug, num_devices=B)

    wb = nc.dram_tensor("wb", [128, WB], F16, kind="ExternalInput")
    # pT columns: [d0h0 | d1h0 | d0h1 | d1h1], 512 each
    pT = nc.dram_tensor("pT", [128, 2 * HW], F16, kind="ExternalInput")
    mk = nc.dram_tensor("mk", [N, HW], F16, kind="ExternalInput")
    outC = nc.dram_tensor("outC", [4, CHUNK], F16, kind="ExternalOutput")

    with tile.TileContext(nc) as tc:
        with (
            nc.allow_low_precision(reason="fp16 matmuls, fp32 PSUM accumulation"),
            tc.tile_pool(name="big", bufs=1) as big,
            tc.tile_pool(name="small", bufs=1) as small,
            tc.tile_pool(name="outp", bufs=4) as outp,
            tc.tile_pool(name="psT", bufs=4, space=bass.MemorySpace.PSUM) as psT,
            tc.tile_pool(name="pstmp", bufs=2, space=bass.MemorySpace.PSUM) as pstmp,
        ):
            # ---- input DMAs: sync + gpsimd rings only (scalar's ring is
            # blocked ~1.3us by its ACT table load)
            wb_sb = big.tile([128, WB], F16)
            nc.sync.dma_start(wb_sb[:], wb[:])
            mk_sb = small.tile([N, HW], F16)
            nc.gpsimd.dma_start(mk_sb[:], mk[:])
            pT_sb = big.tile([128, 2 * HW], F16)
            nc.sync.dma_start(pT_sb[:, 0:HW], pT[:, 0:HW])
            nc.gpsimd.dma_start(pT_sb[:, HW : 2 * HW], pT[:, HW : 2 * HW])

            We_sb = [wb_sb[:, O * k : O * (k + 1)] for k in range(2)]
            eT_sb = [wb_sb[:, 2 * O + N * k : 2 * O + N * (k + 1)] for k in range(2)]
            Wp_sb = [
                wb_sb[:, 2 * O + 2 * N + O * k : 2 * O + 2 * N + O * (k + 1)]
                for k in range(2)
            ]

            # ---- PE warmup: dummy accumulation group on a memset tile.
            wz = small.tile([128, 128], F16)
            nc.vector.memset(wz[:], 0.0)
            wps = pstmp.tile([128, 512], FP, tag="warm")
            for i in range(NWARM):
                nc.tensor.matmul(
                    wps[:, 0:128], wz[:], wz[:],
                    start=(i == 0), stop=(i == NWARM - 1),
                )

            # ---- inj = embs_aug @ We -> [16, 256]
            psumI = pstmp.tile([N, O], FP, tag="inj")
            nc.tensor.matmul(psumI[:], eT_sb[0], We_sb[0], start=True, stop=False)
            nc.tensor.matmul(psumI[:], eT_sb[1], We_sb[1], start=False, stop=True)
            inj_sb = small.tile([N, O], F16)
            nc.vector.tensor_copy(inj_sb[:], psumI[:])

            # ---- main: outT[o,hw] = Wp^T @ pT + inj^T @ maskN, 4 chunks
            copy_eng = ["v", "s", "s", "v"]
            dma_eng = [nc.sync, nc.gpsimd, nc.sync, nc.scalar]
            for k, (oc, hc) in enumerate(GROUPS):
                o0 = 128 * oc
                h0 = 512 * hc
                psum = psT.tile([128, 512], FP, tag="psT")
                nc.tensor.matmul(
                    psum[:], Wp_sb[0][:, o0 : o0 + 128],
                    pT_sb[:, 1024 * hc : 1024 * hc + 512],
                    start=True, stop=False,
                )
                nc.tensor.matmul(
                    psum[:], Wp_sb[1][:, o0 : o0 + 128],
                    pT_sb[:, 1024 * hc + 512 : 1024 * hc + 1024],
                    start=False, stop=False,
                )
                nc.tensor.matmul(
                    psum[:], inj_sb[:, o0 : o0 + 128],
                    mk_sb[:, h0 : h0 + 512],
                    start=False, stop=True,
                )
                och = outp.tile([128, 512], F16, tag="och")
                if copy_eng[k] == "v":
                    nc.vector.tensor_copy(och[:], psum[:])
                else:
                    nc.scalar.activation(och[:], psum[:], AF.Copy)
                dma_eng[k].dma_start(outC[k : k + 1, :], och[:])

    nc.compile()
    return nc


def _host_maskN(locations):
    """Rasterize PATCH_SIZE-rounded boxes + image box, normalize by the
    per-pixel mask count.  [B,15,4] int32 -> [B,16,1024] float32."""
    loc = locations.astype(np.int64)
    starts = loc[..., :2] - loc[..., :2] % 2
    ends = loc[..., 2:] + (2 - loc[..., 2:] % 2)
    rows = np.arange(H)
    cols = np.arange(W)
    rm = (rows[None, None, :] >= starts[..., 0:1]) & (rows[None, None, :] < ends[..., 0:1])
    cm = (cols[None, None, :] >= starts[..., 1:2]) & (cols[None, None, :] < ends[..., 1:2])
    m = (rm[:, :, :, None] & cm[:, :, None, :]).reshape(B, NOBJ, HW).astype(np.float32)
    m = np.concatenate([m, np.ones((B, 1, HW), np.float32)], axis=1)  # [B,16,HW]
    s = m.sum(axis=1, keepdims=True)
    return m / s


def make_in_maps(inputs):
    patches = np.asarray(inputs["patches"], dtype=np.float32)
    embs = np.asarray(inputs["embs"], dtype=np.float32)
    locations = np.asarray(inputs["locations"], dtype=np.int32)
    Wp = np.asarray(inputs["Wp"], dtype=np.float32)
    We = np.asarray(inputs["We"], dtype=np.float32)

    maskN = _host_maskN(locations).astype(np.float16)  # [B,16,1024]
    embs_aug = np.concatenate([embs, embs.mean(axis=1, keepdims=True)], axis=1)
    eT = embs_aug.transpose(0, 2, 1)  # [B,256,16]

    wb_common = np.zeros((128, WB), dtype=np.float16)
    wb_common[:, 0:O] = We[0:128]
    wb_common[:, O : 2 * O] = We[128:256]
    wb_common[:, 2 * O + 2 * N : 3 * O + 2 * N] = Wp[0:128]
    wb_common[:, 3 * O + 2 * N : 4 * O + 2 * N] = Wp[128:256]

    in_maps = []
    for b in range(B):
        wbb = wb_common.copy()
        wbb[:, 2 * O : 2 * O + N] = eT[b, 0:128]
        wbb[:, 2 * O + N : 2 * O + 2 * N] = eT[b, 128:256]
        pTb = patches[b].reshape(HW, D).T.astype(np.float16)  # [256, 1024]
        # columns: [d0h0 | d1h0 | d0h1 | d1h1]
        pT2 = np.concatenate(
            [pTb[0:128, 0:512], pTb[128:256, 0:512],
             pTb[0:128, 512:1024], pTb[128:256, 512:1024]],
            axis=1,
        )
        in_maps.append(
            {
                "wb": wbb,
                "pT": np.ascontiguousarray(pT2),
                "mk": np.ascontiguousarray(maskN[b]),
            }
        )
    return in_maps


_NC = None


def _get_nc():
    global _NC
    if _NC is None:
        _NC = build_nc(debug=False)
    return _NC


def run(inputs, trace: bool = False, **kwargs):
    nc = _get_nc()
    res = bass_utils.run_bass_kernel_spmd(
        nc, make_in_maps(inputs), core_ids=list(range(B)), trace=trace, **kwargs
    )
    full = np.empty((B, HW, O), dtype=np.float32)
    for b in range(B):
        chunks = res.results[b]["outC"].reshape(4, 128, 512)
        outT = np.empty((O, HW), dtype=np.float32)
        for k, (oc, hc) in enumerate(GROUPS):
            outT[128 * oc : 128 * (oc + 1), 512 * hc : 512 * (hc + 1)] = chunks[k]
        full[b] = outT.T
    return full, res


def kernel(**inputs) -> np.ndarray:
    full, _ = run(inputs, trace=False)
    return full
